# revision 1
# baseline (speedup 1.0000x reference)
"""MoE adapter (top-1 of 4 experts, dense all-expert reference) on 8 TRN2 NeuronCores.

Strategy
--------
Data-parallel over the 32768 tokens (4096 per core); expert weights replicated.

The reference computes every expert's bottleneck MLP (D=768 -> H=192 -> D=768)
on all tokens and combines with the one-hot top-1 dispatch mask.  Since
4 experts x H=192 = 768, the four expert MLPs stack into two dense 768x768
matmuls:

    h_all = gelu(x @ W1_stacked + b1_stacked)       # [T, 768]
    mh    = h_all * expand(one_hot)                  # zero non-selected blocks
    y     = mh @ W2_stacked + one_hot @ b2           # [T, 768]
    out   = y + x

(the mask commutes with gelu because it is 0/1 valued).

Everything rides fp16 (vs the earlier bf16 hi/lo version): fp16's 11-bit
mantissa makes a single-pass router accurate enough (a handful of argmax
flips across 32768 tokens, ~2e-3 rel err total vs the 2e-2 gate) and halves
the x / skip / output DMA streams.  All big streams ship in pre-tiled,
partition-contiguous host layouts so every DMA is ~128 large descriptors
(strided layouts cost ~26ns of HWDGE descriptor generation per descriptor
and can starve the router phase).

On-chip layouts avoid all transposes:
  * host ships x two ways: token-major fp16 (skip add) and feature-major
    fp16 (router + matmul moving operand),
  * mm1 produces h feature-major [H, tok] (lhsT = W1 chunk, rhs = x^T),
  * mm2 uses the masked h as the *stationary* operand so y comes out
    token-major [tok, D] and DMAs out (fp16) contiguously.
"""

import numpy as np
import ml_dtypes

import concourse.bass as bass
import concourse.mybir as mybir
import concourse.tile as tile
from concourse import bacc
from concourse.bass_utils import run_bass_kernel_spmd

F16 = np.float16
F32 = np.float32

B, S, D = 16, 2048, 768
H, E = 192, 4
N_CORES = 8
TOK_TOTAL = B * S                 # 32768
TOK = TOK_TOTAL // N_CORES        # 4096 tokens per core
TILE = 512                        # tokens per pipeline tile
N_TILES = TOK // TILE             # 8
SUBT = TILE // 128                # 4 token subtiles of 128
KC = D // 128                     # 6 contraction chunks

_NC_CACHE = None


def _build_bass():
    dt = mybir.dt
    nc = bacc.Bacc("TRN2", target_bir_lowering=False)

    x32 = nc.dram_tensor("x32", [N_TILES, 128, SUBT * D], dt.float16, kind="ExternalInput")
    xht = nc.dram_tensor("xht", [N_TILES, 128, KC * TILE], dt.float16, kind="ExternalInput")
    xh8d = nc.dram_tensor("xh8d", [N_TILES, 128, KC * TILE], dt.float8e4, kind="ExternalInput")
    w1s = nc.dram_tensor("w1s", [128, KC * D], dt.float8e4, kind="ExternalInput")
    w2s = nc.dram_tensor("w2s", [128, KC * D], dt.float8e4, kind="ExternalInput")
    rwhl = nc.dram_tensor("rwhl", [D, E], dt.float16, kind="ExternalInput")
    eexp = nc.dram_tensor("eexp", [E, 128], dt.float16, kind="ExternalInput")
    b2s = nc.dram_tensor("b2s", [E, D], dt.float16, kind="ExternalInput")
    b1r = nc.dram_tensor("b1r", [128, KC], dt.float32, kind="ExternalInput")
    rbt = nc.dram_tensor("rbt", [32, E], dt.float32, kind="ExternalInput")
    out = nc.dram_tensor("out", [TOK, D], dt.float16, kind="ExternalOutput")

    # feature-major x views, pre-tiled partition-contiguous
    xht_r = xht.rearrange("n p (c t) -> n p c t", c=KC)
    xh8_r = xh8d.rearrange("n p (c t) -> n p c t", c=KC)
    x32_r = x32.rearrange("n p (a d) -> n p a d", a=SUBT)

    add = mybir.AluOpType.add
    mult = mybir.AluOpType.mult
    amax = mybir.AluOpType.max
    iseq = mybir.AluOpType.is_equal

    with tile.TileContext(nc) as tc:
        with (
            tc.tile_pool(name="const", bufs=1) as const,
            tc.tile_pool(name="xin", bufs=3) as xin,
            tc.tile_pool(name="hbuf", bufs=3) as hbuf,
            tc.tile_pool(name="obuf", bufs=4) as obuf,
            tc.tile_pool(name="small", bufs=3) as small,
            tc.tile_pool(name="ps_rm", bufs=2, space="PSUM") as ps_rm,
            tc.tile_pool(name="ps_h", bufs=2, space="PSUM") as ps_h,
            tc.tile_pool(name="ps_y", bufs=4, space="PSUM") as ps_y,
        ):
            # Small constants go on the gpsimd (SWDGE) queue; the two big weight
            # matrices ride the sync HWDGE FIFO interleaved with tile-0's loads
            # in exact first-use order (xh, xl, w1, x32, w2).
            rwsb = const.tile([128, KC, E], dt.float16)
            nc.gpsimd.dma_start(rwsb, rwhl.rearrange("(c p) e -> p c e", p=128))
            rbsb = const.tile([32, E], dt.float32)
            nc.gpsimd.dma_start(rbsb, rbt[:])
            b1sb = const.tile([128, KC], dt.float32)
            nc.gpsimd.dma_start(b1sb, b1r[:])
            eesb = const.tile([E, 128], dt.float16)
            nc.gpsimd.dma_start(eesb, eexp[:])
            b2sb = const.tile([E, D], dt.float16)
            nc.gpsimd.dma_start(b2sb, b2s[:])
            w1sb = const.tile([128, KC, D], dt.float8e4)
            w2sb = const.tile([128, KC, D], dt.float8e4)

            def load_tiles(it):
                t0 = it * TILE
                xh = xin.tile([128, KC, TILE], dt.float16, tag="xh")
                nc.sync.dma_start(xh, xht_r[it])
                xh8 = xin.tile([128, KC, TILE], dt.float8e4, tag="xh8")
                nc.sync.dma_start(xh8, xh8_r[it])
                if it == 0:
                    nc.sync.dma_start(w1sb, w1s.rearrange("p (c h) -> p c h", c=KC))
                x32t = xin.tile([128, SUBT, D], dt.float16, tag="x32t")
                nc.sync.dma_start(x32t, x32_r[it])
                if it == 0:
                    nc.sync.dma_start(w2sb, w2s.rearrange("p (c h) -> p c h", c=KC))
                return xh, xh8, x32t

            def router_onehot(xh, between=None):
                """logits^T in psum -> one-hot mask mt32[0:4] [4, TILE] fp16."""
                psrm = ps_rm.tile([128, TILE], dt.float32, tag="psrm")
                psr = psrm[0:4]
                for kc in range(KC):
                    nc.tensor.matmul(
                        psr, rwsb[:, kc, :], xh[:, kc, :],
                        start=(kc == 0), stop=(kc == KC - 1),
                    )
                if between is not None:
                    between()
                # alignment-safe one-hot argmax via DVE 32x32 stream transpose
                lt32s = small.tile([32, TILE], dt.float32, tag="lt32s")
                nc.scalar.copy(lt32s[0:4], psr)
                # token-major blocks: lt32[p, 32g+r] = lt32s[r, 32g+p]
                lt32 = small.tile([32, TILE], dt.float32, tag="lt32")
                nc.vector.transpose(lt32, lt32s)
                v = lt32.rearrange("p (g r) -> p g r", r=32)
                lt_tok = small.tile([32, TILE // 32, E], dt.float32, tag="lt_tok")
                nc.vector.tensor_tensor(
                    lt_tok, v[:, :, 0:E],
                    rbsb[:, None, :].to_broadcast((32, TILE // 32, E)), add,
                )
                mxg = small.tile([32, TILE // 32], dt.float32, tag="mxg")
                nc.vector.tensor_reduce(
                    out=mxg, in_=lt_tok, axis=mybir.AxisListType.X, op=amax
                )
                mtb = small.tile([32, TILE], dt.float16, tag="mtb")
                mview = mtb.rearrange("p (g r) -> p g r", r=32)
                nc.vector.tensor_tensor(
                    mview[:, :, 0:4], lt_tok,
                    mxg[:, :, None].to_broadcast((32, TILE // 32, E)), iseq,
                )
                # back-transpose: mt32[e, t] = one_hot[t, e] for e < 4
                mt32 = small.tile([32, TILE], dt.float16, tag="mt32")
                nc.vector.transpose(mt32, mtb)
                return mt32, psrm

            # ---- PE warm-up burst: spin the HAM up to K=8/8 during the DMA head
            dummy = const.tile([128, TILE], dt.float16)
            nc.vector.memset(dummy, 0.0)
            psd = ps_h.tile([128, TILE], dt.float32, tag="psh")
            for _ in range(10):
                nc.tensor.matmul(psd, dummy[:, 0:128], dummy, start=True, stop=True)

            # software pipeline: router/one-hot for tile n+1 issues at the end of
            # iteration n, so the mask chain latency hides under mm1/mm2.
            # Loads run two tiles ahead.
            tiles = {0: load_tiles(0)}
            mt32, psm_next = router_onehot(tiles[0][0])
            tiles[1] = load_tiles(1)

            for it in range(N_TILES):
                t0 = it * TILE
                mt = mt32[0:4]
                xh, xh8, x32t = tiles[it]

                if it + 2 < N_TILES:
                    tiles[it + 2] = load_tiles(it + 2)

                # ---- mm1: h^T = gelu(W1^T x + b1), then mask ----
                # experts are interleaved along H (unit j of expert e at 4j+e),
                # so the expanded one-hot is the same [128, TILE] tile for every
                # H-chunk: a single K=4 matmul per tile, reusing the router's
                # psum bank (its logit rows were already copied to SBUF).
                psm = psm_next
                nc.tensor.matmul(psm, eesb, mt, start=True, stop=True)
                psm_sb = hbuf.tile([128, TILE], dt.float16, tag="psm_sb")
                nc.scalar.copy(psm_sb, psm)
                mh = hbuf.tile([128, KC, TILE], dt.float8e4, tag="mh")
                hchunk = hbuf.tile([128, KC, TILE], dt.float16, tag="hchunk")
                for hc in range(KC):
                    psh = ps_h.tile([128, TILE], dt.float32, tag="psh")
                    for k2 in range(KC // 2):
                        nc.tensor.matmul(
                            psh,
                            w1sb[:, 2 * k2 : 2 * k2 + 2, hc * 128 : (hc + 1) * 128],
                            xh8[:, 2 * k2 : 2 * k2 + 2, :],
                            start=(k2 == 0), stop=(k2 == KC // 2 - 1),
                            perf_mode=mybir.MatmulPerfMode.DoubleRow,
                        )
                    # W1 is shipped x16 (fp8 subnormal safety): descale here
                    nc.scalar.activation(
                        hchunk[:, hc, :], psh,
                        mybir.ActivationFunctionType.Gelu,
                        bias=b1sb[:, hc : hc + 1], scale=1.0 / 16.0,
                    )
                    eng = nc.vector if hc in (0, 1, 5) else nc.gpsimd
                    eng.tensor_tensor(
                        mh[:, hc, :], hchunk[:, hc, :], psm_sb, mult
                    )

                # ---- mm2: y = mh^T.T @ W2 + one_hot @ b2, token-major ----
                if it + 1 < N_TILES:
                    mt32_next, psm_next = router_onehot(tiles[it + 1][0])
                out_r = out[t0 : t0 + TILE].rearrange("(a p) d -> p a d", p=128)
                for a in range(SUBT):
                    osb = obuf.tile([128, D], dt.float16, tag="osb")
                    # interleave the two half-D psum groups, b2 first: the MMs
                    # that consume the freshest mh chunk land late in the group,
                    # hiding the gelu+mask chain of mm1's last chunk.
                    psy = [ps_y.tile([128, 384], dt.float32, tag="psy",
                                     name=f"psy_{it}_{a}_{h}")
                           for h in range(2)]
                    for half in range(2):
                        nc.tensor.matmul(
                            psy[half],
                            mt[:, a * 128 : (a + 1) * 128],
                            b2sb[:, half * 384 : half * 384 + 384],
                            start=True, stop=False,
                        )
                    for k2 in range(KC // 2):
                        for half in range(2):
                            nc.tensor.matmul(
                                psy[half],
                                mh[:, 2 * k2 : 2 * k2 + 2, a * 128 : (a + 1) * 128],
                                w2sb[:, 2 * k2 : 2 * k2 + 2,
                                     half * 384 : half * 384 + 384],
                                start=False, stop=(k2 == KC // 2 - 1),
                                perf_mode=mybir.MatmulPerfMode.DoubleRow,
                            )
                    for half in range(2):
                        d0 = half * 384
                        nc.vector.tensor_tensor(
                            osb[:, d0 : d0 + 384], psy[half],
                            x32t[:, a, d0 : d0 + 384], add,
                        )
                    # per-subtile store on the ACT HWDGE ring (doesn't block loads)
                    nc.scalar.dma_start(out_r[:, a, :], osb)

                if it + 1 < N_TILES:
                    mt32 = mt32_next
                del tiles[it]

    nc.compile()
    return nc


def _prep_inputs(x, router_w, router_b, w1, b1, w2, b2):
    """Host-side packing: cast/transpose; returns per-core input dicts."""
    xf = np.ascontiguousarray(np.asarray(x, dtype=F32).reshape(TOK_TOTAL, D))
    x_hi = xf.astype(F16)

    rw = np.asarray(router_w, dtype=F32)
    rwhl = np.ascontiguousarray(rw.astype(F16))  # [D, 4]

    w1f = np.asarray(w1, dtype=F32)           # [E, D, H]
    w2f = np.asarray(w2, dtype=F32)           # [E, H, D]
    b1f = np.asarray(b1, dtype=F32)           # [E, H]
    b2f = np.asarray(b2, dtype=F32)           # [E, D]
    rb = np.asarray(router_b, dtype=F32)      # [E]

    # experts interleaved along the stacked hidden dim: unit j of expert e
    # lives at index 4j + e  -> the one-hot expansion pattern repeats every
    # 4 partitions, identically for each 128-row chunk.
    # Weights pre-arranged partition-contiguous: [p, c*D+m] = W[(c*128+p), m]
    w1st = w1f.transpose(1, 2, 0).reshape(D, H * E).astype(F16)
    w2st = w2f.transpose(1, 0, 2).reshape(H * E, D).astype(F16)
    F8 = ml_dtypes.float8_e4m3
    w1s = np.ascontiguousarray(
        (w1st.astype(F32) * 16.0)
        .reshape(KC, 128, D).transpose(1, 0, 2).reshape(128, KC * D)).astype(F8)
    w2s = np.ascontiguousarray(
        (w2st.astype(F32) * 16.0)
        .reshape(KC, 128, D).transpose(1, 0, 2).reshape(128, KC * D)).astype(F8)
    b1all = np.ascontiguousarray(b1f.T.reshape(E * H))                    # [768]
    b1r = np.ascontiguousarray(b1all.reshape(KC, 128).T).astype(F32)      # [128, 6]
    b2sb = (b2f * 16.0).astype(F16)
    rbt = np.ascontiguousarray(np.tile(rb.reshape(1, E), (32, 1))).astype(F32)

    ee = np.zeros((E, 128), dtype=F16)
    for e in range(E):
        ee[e, e::E] = 1

    in_maps = []
    for c in range(N_CORES):
        sl = slice(c * TOK, (c + 1) * TOK)
        xc = x_hi[sl]
        # token-major skip stream, pre-tiled: [tile, p, (a d)]
        x32_t = np.ascontiguousarray(
            (xc.astype(F32) * 16.0).astype(F16)
            .reshape(N_TILES, SUBT, 128, D).transpose(0, 2, 1, 3)
            .reshape(N_TILES, 128, SUBT * D))
        # feature-major stream, pre-tiled: [tile, p, (c t)]
        xht_t = np.ascontiguousarray(
            xc.T.reshape(KC, 128, N_TILES, TILE)
            .transpose(2, 1, 0, 3).reshape(N_TILES, 128, KC * TILE))
        xh8_t = np.ascontiguousarray(xht_t.astype(ml_dtypes.float8_e4m3))
        in_maps.append(
            {
                "x32": x32_t,
                "xht": xht_t,
                "xh8d": xh8_t,
                "w1s": w1s,
                "w2s": w2s,
                "rwhl": rwhl,
                "eexp": ee,
                "b2s": b2sb,
                "b1r": b1r,
                "rbt": rbt,
            }
        )
    return in_maps


def _get_nc():
    global _NC_CACHE
    if _NC_CACHE is None:
        _NC_CACHE = _build_bass()
    return _NC_CACHE


def kernel(x, router_w, router_b, w1, b1, w2, b2, _trace=False, _trace_kwargs=None):
    in_maps = _prep_inputs(x, router_w, router_b, w1, b1, w2, b2)
    nc = _get_nc()
    res = run_bass_kernel_spmd(
        nc,
        in_maps,
        core_ids=list(range(N_CORES)),
        trace=_trace,
        **(_trace_kwargs or {}),
    )
    outs = [r["out"].astype(np.float32) * (1.0 / 16.0) for r in res.results]
    full = np.concatenate(outs, axis=0).reshape(B, S, D)
    if _trace:
        kernel.last_results = res
    return full



# revision 2
# speedup vs baseline: 1.0189x; 1.0189x over previous
"""MoE adapter (top-1 of 4 experts, dense all-expert reference) on 8 TRN2 NeuronCores.

Strategy
--------
Data-parallel over the 32768 tokens (4096 per core); expert weights replicated.

The reference computes every expert's bottleneck MLP (D=768 -> H=192 -> D=768)
on all tokens and combines with the one-hot top-1 dispatch mask.  Since
4 experts x H=192 = 768, the four expert MLPs stack into two dense 768x768
matmuls:

    h_all = gelu(x @ W1_stacked + b1_stacked)       # [T, 768]
    mh    = h_all * expand(one_hot)                  # zero non-selected blocks
    y     = mh @ W2_stacked + one_hot @ b2           # [T, 768]
    out   = y + x

(the mask commutes with gelu because it is 0/1 valued).

Everything is feature-major on chip: a single fp16 stream of 16*x (feature-
major, pre-tiled partition-contiguous) feeds the router, the skip connection
AND (via a host-side fp8 shadow copy) mm1's moving operand.  mm2 keeps W2 as
the *stationary* operand (resident in SBUF all kernel long, so every
LDWEIGHTS is pulled ahead by the PE reorder window) and produces y^T
feature-major, the skip add reuses the same x^T tile, and the output ships
feature-major (the host untransposes).  This removes the token-major x
stream of the earlier version entirely (-6.3MB/core DMA) and removes all
just-in-time stationary loads from the PE critical path.

Scaling: x ships as 16*x (argmax is scale invariant given a 16*router_bias),
W1/W2/b2 ship as 16*w (fp8 subnormal safety), so mm1 psum = 256*(x@w1)
(descaled inside the gelu activation), mm2 psum = 16*y, the skip add
produces 16*(y+x) in fp16 and the host divides by 16.
"""

import numpy as np
import ml_dtypes

import concourse.bass as bass
import concourse.mybir as mybir
import concourse.tile as tile
from concourse import bacc
from concourse.bass_utils import run_bass_kernel_spmd

F16 = np.float16
F32 = np.float32

B, S, D = 16, 2048, 768
H, E = 192, 4
N_CORES = 8
TOK_TOTAL = B * S                 # 32768
TOK = TOK_TOTAL // N_CORES        # 4096 tokens per core
TILE = 512                        # tokens per pipeline tile
N_TILES = TOK // TILE             # 8
KC = D // 128                     # 6 contraction chunks

_NC_CACHE = None


def _build_bass():
    dt = mybir.dt
    nc = bacc.Bacc("TRN2", target_bir_lowering=False)

    xht = nc.dram_tensor("xht", [N_TILES, 128, KC * TILE], dt.float16, kind="ExternalInput")
    xh8d = nc.dram_tensor("xh8d", [N_TILES, 128, KC * TILE], dt.float8e4, kind="ExternalInput")
    w1s = nc.dram_tensor("w1s", [128, KC * D], dt.float8e4, kind="ExternalInput")
    w2s = nc.dram_tensor("w2s", [128, KC * D], dt.float8e4, kind="ExternalInput")
    rwhl = nc.dram_tensor("rwhl", [D, E], dt.float16, kind="ExternalInput")
    eexp = nc.dram_tensor("eexp", [E, 128], dt.float16, kind="ExternalInput")
    b2s = nc.dram_tensor("b2s", [E, D], dt.float16, kind="ExternalInput")
    b1r = nc.dram_tensor("b1r", [128, KC], dt.float32, kind="ExternalInput")
    rbt = nc.dram_tensor("rbt", [32, E], dt.float32, kind="ExternalInput")
    out = nc.dram_tensor("out", [N_TILES, 128, KC * TILE], dt.float16, kind="ExternalOutput")

    # feature-major x views, pre-tiled partition-contiguous
    xht_r = xht.rearrange("n p (c t) -> n p c t", c=KC)
    xh8_r = xh8d.rearrange("n p (c t) -> n p c t", c=KC)
    out_r = out.rearrange("n p (c t) -> n p c t", c=KC)

    add = mybir.AluOpType.add
    mult = mybir.AluOpType.mult
    amax = mybir.AluOpType.max
    iseq = mybir.AluOpType.is_equal

    with tile.TileContext(nc) as tc:
        with (
            tc.tile_pool(name="const", bufs=1) as const,
            tc.tile_pool(name="xin", bufs=3) as xin,
            tc.tile_pool(name="hbuf", bufs=3) as hbuf,
            tc.tile_pool(name="obuf", bufs=2) as obuf,
            tc.tile_pool(name="small", bufs=3) as small,
            tc.tile_pool(name="ps_rm", bufs=2, space="PSUM") as ps_rm,
            tc.tile_pool(name="ps_h", bufs=2, space="PSUM") as ps_h,
            tc.tile_pool(name="ps_y", bufs=4, space="PSUM") as ps_y,
        ):
            # ---- constants.  The router weight rides the sync HWDGE FIFO
            # FIRST (it gates tile-0's router matmuls); the other small
            # constants go on the scalar (ACT) HWDGE ring which is otherwise
            # idle during the DMA head.  Nothing latency-critical rides the
            # slow SWDGE path.
            rwsb = const.tile([128, KC, E], dt.float16)
            nc.sync.dma_start(rwsb, rwhl.rearrange("(c p) e -> p c e", p=128))
            eesb = const.tile([E, 128], dt.float16)
            nc.scalar.dma_start(eesb, eexp[:])
            rbsb = const.tile([32, E], dt.float32)
            nc.scalar.dma_start(rbsb, rbt[:])
            b1sb = const.tile([128, KC], dt.float32)
            nc.scalar.dma_start(b1sb, b1r[:])
            b2sb = const.tile([E, D], dt.float16)
            nc.scalar.dma_start(b2sb, b2s[:])
            w1sb = const.tile([128, KC, D], dt.float8e4)
            w2sb = const.tile([128, KC, D], dt.float8e4)

            def load_tiles(it):
                xh = xin.tile([128, KC, TILE], dt.float16, tag="xh")
                nc.sync.dma_start(xh, xht_r[it])
                if it == 0:
                    nc.sync.dma_start(w1sb, w1s.rearrange("p (c h) -> p c h", c=KC))
                xh8 = xin.tile([128, KC, TILE], dt.float8e4, tag="xh8")
                nc.sync.dma_start(xh8, xh8_r[it])
                if it == 1:
                    nc.sync.dma_start(w2sb, w2s.rearrange("p (c h) -> p c h", c=KC))
                return xh, xh8

            def router_onehot(xh):
                """logits^T in psum -> one-hot mask mt32[0:4] [4, TILE] fp16."""
                psrm = ps_rm.tile([128, TILE], dt.float32, tag="psrm")
                psr = psrm[0:4]
                for kc in range(KC):
                    nc.tensor.matmul(
                        psr, rwsb[:, kc, :], xh[:, kc, :],
                        start=(kc == 0), stop=(kc == KC - 1),
                    )
                # alignment-safe one-hot argmax via DVE 32x32 stream transpose
                lt32s = small.tile([32, TILE], dt.float32, tag="lt32s")
                nc.scalar.copy(lt32s[0:4], psr)
                # token-major blocks: lt32[p, 32g+r] = lt32s[r, 32g+p]
                lt32 = small.tile([32, TILE], dt.float32, tag="lt32")
                nc.vector.transpose(lt32, lt32s)
                v = lt32.rearrange("p (g r) -> p g r", r=32)
                lt_tok = small.tile([32, TILE // 32, E], dt.float32, tag="lt_tok")
                nc.vector.tensor_tensor(
                    lt_tok, v[:, :, 0:E],
                    rbsb[:, None, :].to_broadcast((32, TILE // 32, E)), add,
                )
                mxg = small.tile([32, TILE // 32], dt.float32, tag="mxg")
                nc.vector.tensor_reduce(
                    out=mxg, in_=lt_tok, axis=mybir.AxisListType.X, op=amax
                )
                mtb = small.tile([32, TILE], dt.float16, tag="mtb")
                mview = mtb.rearrange("p (g r) -> p g r", r=32)
                nc.vector.tensor_tensor(
                    mview[:, :, 0:4], lt_tok,
                    mxg[:, :, None].to_broadcast((32, TILE // 32, E)), iseq,
                )
                # back-transpose: mt32[e, t] = one_hot[t, e] for e < 4
                mt32 = small.tile([32, TILE], dt.float16, tag="mt32")
                nc.vector.transpose(mt32, mtb)
                return mt32, psrm

            # ---- PE warm-up burst: spin the HAM toward K=8/8 during the DMA
            # head so the real matmuls start at full clock.
            dummy = const.tile([128, TILE], dt.float16)
            nc.vector.memset(dummy, 0.0)
            psd = ps_h.tile([128, TILE], dt.float32, tag="psh")
            for _ in range(8):
                nc.tensor.matmul(psd, dummy[:, 0:128], dummy, start=True, stop=True)

            # software pipeline: router/one-hot for tile n+1 issues between
            # mm1 and mm2 of tile n, so the mask chain latency hides under
            # the mm2 matmuls.  Loads run two tiles ahead.
            tiles = {0: load_tiles(0)}
            mt32, psm_next = router_onehot(tiles[0][0])
            tiles[1] = load_tiles(1)

            for it in range(N_TILES):
                mt = mt32[0:4]
                xh, xh8 = tiles[it]

                if it + 2 < N_TILES:
                    tiles[it + 2] = load_tiles(it + 2)

                # ---- mm1: h^T = gelu(W1^T x + b1), then mask ----
                # experts are interleaved along H (unit j of expert e at 4j+e),
                # so the expanded one-hot is the same [128, TILE] tile for every
                # H-chunk: a single K=4 matmul per tile, reusing the router's
                # psum bank (its logit rows were already copied to SBUF).
                psm = psm_next
                nc.tensor.matmul(psm, eesb, mt, start=True, stop=True)
                psm_sb = hbuf.tile([128, TILE], dt.float16, tag="psm_sb")
                nc.scalar.copy(psm_sb, psm)
                mh = hbuf.tile([128, KC, TILE], dt.float8e4, tag="mh")
                hchunk = hbuf.tile([128, KC, TILE], dt.float16, tag="hchunk")
                for hc in range(KC):
                    psh = ps_h.tile([128, TILE], dt.float32, tag="psh")
                    for k2 in range(KC // 2):
                        nc.tensor.matmul(
                            psh,
                            w1sb[:, 2 * k2 : 2 * k2 + 2, hc * 128 : (hc + 1) * 128],
                            xh8[:, 2 * k2 : 2 * k2 + 2, :],
                            start=(k2 == 0), stop=(k2 == KC // 2 - 1),
                            perf_mode=mybir.MatmulPerfMode.DoubleRow,
                        )
                    # x and W1 both shipped x16: descale 1/256 here
                    nc.scalar.activation(
                        hchunk[:, hc, :], psh,
                        mybir.ActivationFunctionType.Gelu,
                        bias=b1sb[:, hc : hc + 1], scale=1.0 / 256.0,
                    )
                    eng = nc.gpsimd if hc in (2, 3) else nc.vector
                    eng.tensor_tensor(
                        mh[:, hc, :], hchunk[:, hc, :], psm_sb, mult
                    )

                # router for the next tile: its 6 fp16 matmuls sit between
                # mm1's and mm2's matmuls in the PE queue, buying time for
                # the gelu+mask chain of mm1's last chunk.
                if it + 1 < N_TILES:
                    mt32_next, psm_next = router_onehot(tiles[it + 1][0])

                # ---- mm2: y^T = W2s^T @ mh + b2^T @ one_hot, feature-major.
                # W2 chunks are the *stationary* operands (resident in SBUF),
                # mh streams; output chunk dc is 128 features x TILE tokens.
                osb = obuf.tile([128, KC, TILE], dt.float16, tag="osb")
                for dc in range(KC):
                    psy = ps_y.tile([128, TILE], dt.float32, tag="psy")
                    nc.tensor.matmul(
                        psy, b2sb[:, dc * 128 : (dc + 1) * 128], mt,
                        start=True, stop=False,
                    )
                    for k2 in range(KC // 2):
                        nc.tensor.matmul(
                            psy,
                            w2sb[:, 2 * k2 : 2 * k2 + 2, dc * 128 : (dc + 1) * 128],
                            mh[:, 2 * k2 : 2 * k2 + 2, :],
                            start=False, stop=(k2 == KC // 2 - 1),
                            perf_mode=mybir.MatmulPerfMode.DoubleRow,
                        )
                    # skip connection: x^T rides the same feature-major tile
                    nc.vector.tensor_tensor(
                        osb[:, dc, :], psy, xh[:, dc, :], add,
                    )
                    # store per half-tile on the ACT HWDGE ring
                    if dc == KC // 2 - 1:
                        nc.scalar.dma_start(
                            out_r[it, :, 0 : KC // 2, :], osb[:, 0 : KC // 2, :]
                        )
                    elif dc == KC - 1:
                        nc.scalar.dma_start(
                            out_r[it, :, KC // 2 : KC, :], osb[:, KC // 2 : KC, :]
                        )

                if it + 1 < N_TILES:
                    mt32 = mt32_next
                del tiles[it]

    nc.compile()
    return nc


def _prep_inputs(x, router_w, router_b, w1, b1, w2, b2):
    """Host-side packing: cast/transpose; returns per-core input dicts."""
    xf = np.ascontiguousarray(np.asarray(x, dtype=F32).reshape(TOK_TOTAL, D))

    rw = np.asarray(router_w, dtype=F32)
    rwhl = np.ascontiguousarray(rw.astype(F16))  # [D, 4]

    w1f = np.asarray(w1, dtype=F32)           # [E, D, H]
    w2f = np.asarray(w2, dtype=F32)           # [E, H, D]
    b1f = np.asarray(b1, dtype=F32)           # [E, H]
    b2f = np.asarray(b2, dtype=F32)           # [E, D]
    rb = np.asarray(router_b, dtype=F32)      # [E]

    # experts interleaved along the stacked hidden dim: unit j of expert e
    # lives at index 4j + e  -> the one-hot expansion pattern repeats every
    # 4 partitions, identically for each 128-row chunk.
    # Weights pre-arranged partition-contiguous: [p, c*D+m] = W[(c*128+p), m]
    w1st = w1f.transpose(1, 2, 0).reshape(D, H * E).astype(F16)
    w2st = w2f.transpose(1, 0, 2).reshape(H * E, D).astype(F16)
    F8 = ml_dtypes.float8_e4m3
    w1s = np.ascontiguousarray(
        (w1st.astype(F32) * 16.0)
        .reshape(KC, 128, D).transpose(1, 0, 2).reshape(128, KC * D)).astype(F8)
    w2s = np.ascontiguousarray(
        (w2st.astype(F32) * 16.0)
        .reshape(KC, 128, D).transpose(1, 0, 2).reshape(128, KC * D)).astype(F8)
    b1all = np.ascontiguousarray(b1f.T.reshape(E * H))                    # [768]
    b1r = np.ascontiguousarray(b1all.reshape(KC, 128).T).astype(F32)      # [128, 6]
    b2sb = (b2f * 16.0).astype(F16)
    rbt = np.ascontiguousarray(np.tile(16.0 * rb.reshape(1, E), (32, 1))).astype(F32)

    ee = np.zeros((E, 128), dtype=F16)
    for e in range(E):
        ee[e, e::E] = 1

    in_maps = []
    for c in range(N_CORES):
        sl = slice(c * TOK, (c + 1) * TOK)
        xc = xf[sl]
        # feature-major 16*x stream, pre-tiled: [tile, p, (c t)]
        xht_t = np.ascontiguousarray(
            (xc * 16.0).astype(F16)
            .T.reshape(KC, 128, N_TILES, TILE)
            .transpose(2, 1, 0, 3).reshape(N_TILES, 128, KC * TILE))
        xh8_t = np.ascontiguousarray(xht_t.astype(ml_dtypes.float8_e4m3))
        in_maps.append(
            {
                "xht": xht_t,
                "xh8d": xh8_t,
                "w1s": w1s,
                "w2s": w2s,
                "rwhl": rwhl,
                "eexp": ee,
                "b2s": b2sb,
                "b1r": b1r,
                "rbt": rbt,
            }
        )
    return in_maps


def _get_nc():
    global _NC_CACHE
    if _NC_CACHE is None:
        _NC_CACHE = _build_bass()
    return _NC_CACHE


def kernel(x, router_w, router_b, w1, b1, w2, b2, _trace=False, _trace_kwargs=None):
    in_maps = _prep_inputs(x, router_w, router_b, w1, b1, w2, b2)
    nc = _get_nc()
    res = run_bass_kernel_spmd(
        nc,
        in_maps,
        core_ids=list(range(N_CORES)),
        trace=_trace,
        **(_trace_kwargs or {}),
    )
    outs = []
    for r in res.results:
        o = r["out"].reshape(N_TILES, 128, KC, TILE)     # [it, p, dc, t] fp16
        # y^T feature-major -> token-major [TOK, D], descale by 16
        yt = o.transpose(0, 3, 2, 1).reshape(TOK, D)     # [it*t, dc*128(=d)]
        outs.append(yt.astype(np.float32) * (1.0 / 16.0))
    full = np.concatenate(outs, axis=0).reshape(B, S, D)
    if _trace:
        kernel.last_results = res
    return full
